# revision 2
# baseline (speedup 1.0000x reference)
"""Multi-head attention (B=2, S=2048, D=1024, H=16) on 8 TRN2 NeuronCores, v2.

Sharding: core c handles batch b = c//4 and head group hg = c%4 (4 heads,
256 features f0 = hg*256). Host sums the 4 partial out-projections per batch
and adds bo.

v2 design (vs v1): the ACT engine (exp) is the steady-state bottleneck
(256 x [128,512] exps ~ 183us); the PE p-state drops to 1.2GHz for 3us after
any idle gap. So:
 - exp tiles span 2 PSUM banks ([128, 2, 512]) halving ACT instruction
   overhead: 128 x ~1140ns = ~146us.
 - AV is P-stationary: lhsT = pexp[k128, q128] (weights), rhs = V columns
   plus a ones column [k128, 65]. Full 128-row contraction (no zero padding)
   and the softmax denominator lands in psum column 64 per q-partition, so
   normalization is a per-partition approx-reciprocal + tensor_scalar
   multiply. AV stream: 1024 matmuls x 65 cols = ~28us (was ~55us with
   zero-padded V_aug + ones-broadcast matmuls + big reciprocals).
 - O[q, e] -> OT[e, q] via DMA-transpose (sync engine issues; no PE
   transposes, no identity, no extra psum traffic).
 - Projection psum chains share the scores' 2-bank psum pool (one tag), so
   PSUM = 4 (scores) + 2 (psav) + 2 (out-proj) = 8 banks exactly.
 - Single woven PE instruction stream: scores tiles are paced by ACT
   (~1140ns each); between score tiles the weaver emits ~700ns chunks of
   projection / AV / out-proj work so the PE never idles (keeps max
   p-state) and ACT never starves.
"""
import numpy as np

import concourse.bass as bass
import concourse.mybir as mybir
import concourse.tile as tile
from concourse import bacc
from concourse import bass_utils

F32 = mybir.dt.float32
F16 = mybir.dt.float16
EXP = mybir.ActivationFunctionType.Exp
ADD = mybir.AluOpType.add
MULT = mybir.AluOpType.mult

B, S, D, H = 2, 2048, 1024, 16
HD = D // H          # 64
E = 256              # local features per core (4 heads)
QC = 512             # q-chunk (scores free dim)
N_QC = S // QC       # 4
N_KT = S // 128      # 16 k-tiles
N_ST = S // 128      # 16 s-tiles
KD = D // 128        # 8 contraction tiles for the projections


def build():
    nc = bacc.Bacc("TRN2", target_bir_lowering=False, debug=False, num_devices=8)

    xT = nc.dram_tensor("xT", [D, S], F16, kind="ExternalInput").ap()
    wqT = nc.dram_tensor("wqT", [D, E], F16, kind="ExternalInput").ap()
    wkT = nc.dram_tensor("wkT", [D, E], F16, kind="ExternalInput").ap()
    wvT = nc.dram_tensor("wvT", [D, E], F16, kind="ExternalInput").ap()
    woT = nc.dram_tensor("woT", [E, D], F16, kind="ExternalInput").ap()
    bq2 = nc.dram_tensor("bq2", [128, 2], F32, kind="ExternalInput").ap()
    bk2 = nc.dram_tensor("bk2", [128, 2], F32, kind="ExternalInput").ap()
    bvb = nc.dram_tensor("bvb", [128, E], F32, kind="ExternalInput").ap()

    y = nc.dram_tensor("y", [S, D], F16, kind="ExternalOutput").ap()

    with tile.TileContext(nc) as tc:
        with (
            tc.tile_pool(name="pers", bufs=1) as pp,
            tc.tile_pool(name="pexp_pool", bufs=3) as pxp,
            tc.tile_pool(name="opair_pool", bufs=10) as opp,
            tc.tile_pool(name="rec_pool", bufs=4) as rcp,
            tc.tile_pool(name="ysb_pool", bufs=2) as ysp,
            # scores need an exclusive double-buffered rotation: sharing a
            # tag with the chain tiles makes each scores tile reuse the
            # previous scores tile's banks, serializing PE<->ACT.
            tc.tile_pool(name="ps_sc", bufs=2, space="PSUM") as psb,
            tc.tile_pool(name="ps_chain", bufs=1, space="PSUM") as psc,
            tc.tile_pool(name="ps_av", bufs=2, space="PSUM") as psv,
        ):
            # ---------------- persistent tiles ----------------
            xT_sb = pp.tile([128, KD, S], F16)
            wq_sb = pp.tile([128, KD, E], F16)
            wk_sb = pp.tile([128, KD, E], F16)
            wv_sb = pp.tile([128, KD, E], F16)
            woT_sb = pp.tile([128, 2, D], F16)
            bq_sb = pp.tile([128, 2], F32)
            bk_sb = pp.tile([128, 2], F32)
            bvb_sb = pp.tile([128, E], F32)
            QT_sb = pp.tile([128, 2, S], F16)
            KT_sb = pp.tile([128, 2, S], F16)
            OT_sb = pp.tile([128, 2, S], F16)
            # V plus ones column, per (kt, head): [k128, 65]
            von_sb = pp.tile([128, N_KT, 4, 65], F16)

            # ---------------- preamble DMAs ----------------
            # One big DMA per weight tensor (descriptor-gen cost ~3us on the
            # issuing engine, but the transfer completes ~10us sooner than a
            # serialized per-slice train).  wk on vector and wq on sync run
            # concurrently; xT on gpsimd in consumption order; late-needed
            # weights after.
            nc.scalar.dma_start(
                wk_sb[:], wkT.rearrange("(k p) e -> p k e", p=128))
            nc.sync.dma_start(
                wq_sb[:], wqT.rearrange("(k p) e -> p k e", p=128))
            for sc in range(S // 512):
                for kh in range(2):
                    nc.gpsimd.dma_start(
                        xT_sb[:, kh * 4:(kh + 1) * 4, sc * 512:(sc + 1) * 512],
                        xT[kh * 512:(kh + 1) * 512, sc * 512:(sc + 1) * 512]
                        .rearrange("(k p) c -> p k c", p=128))
            nc.sync.dma_start(bk_sb[:], bk2)
            nc.sync.dma_start(bq_sb[:], bq2)
            nc.sync.dma_start(
                wv_sb[:], wvT.rearrange("(k p) e -> p k e", p=128))
            nc.sync.dma_start(bvb_sb[:], bvb)
            nc.gpsimd.memset(von_sb[:, :, :, 64], 1.0)
            for c in range(2):
                nc.sync.dma_start(woT_sb[:, c, :], woT[c * 128:(c + 1) * 128, :])

            # ---------------- emit helpers (cost-yielding generators) -----
            def proj_chain(sc, w_sb, b_sb, out_sb):
                """One Q/K projection s-block: 2 chains (ec 0/1) through one
                2-bank psum tile. Yields est-ns after every few matmuls."""
                ps = psc.tile([128, 2, QC], F32, tag="chain", name="ps_proj")
                ssl = slice(sc * 512, (sc + 1) * 512)
                yield 500  # spacing: let a scores tile separate chain WARs
                for ec in range(2):
                    for k in range(KD):
                        nc.tensor.matmul(
                            ps[:, ec, :],
                            w_sb[:, k, ec * 128:(ec + 1) * 128],
                            xT_sb[:, k, ssl],
                            start=(k == 0), stop=(k == KD - 1))
                        if k % 3 == 2:
                            yield 650
                    nc.vector.tensor_scalar(
                        out_sb[:, ec, ssl], ps[:, ec, :],
                        b_sb[:, ec:ec + 1], None, ADD)
                    yield 220

            def vproj_pair(stp):
                """V projection for s-tiles 2*stp, 2*stp+1 through one
                2-bank psum tile; scatter+bias into von_sb."""
                ps = psc.tile([128, 2, QC], F32, tag="chain", name="ps_vproj")
                yield 500
                for i in range(2):
                    st = stp * 2 + i
                    for k in range(KD):
                        nc.tensor.matmul(
                            ps[:, i, 0:E],
                            xT_sb[:, k, st * 128:(st + 1) * 128],
                            wv_sb[:, k, :],
                            start=(k == 0), stop=(k == KD - 1))
                        if k % 3 == 2:
                            yield 320
                    nc.vector.tensor_tensor(
                        von_sb[:, st, :, 0:64],
                        ps[:, i, 0:E].rearrange("p (h d) -> p h d", h=4),
                        bvb_sb.rearrange("p (h d) -> p h d", h=4), ADD)
                    yield 210

            opairs = {}

            def av_unit(qc, pr, h, qt, pexp_t):
                """AV for one (head, q-tile of 128): 16 accumulating matmuls
                + normalize into O_pair; after h==1, DMA-transpose to OT."""
                h_abs = pr * 2 + h
                psav = psv.tile([128, 65], F32, tag="av", name="psav")
                qls = slice(qt * 128, (qt + 1) * 128)
                for kt in range(N_KT):
                    nc.tensor.matmul(
                        psav[:], pexp_t[:, kt, qls],
                        von_sb[:, kt, h_abs, :],
                        start=(kt == 0), stop=(kt == N_KT - 1))
                rec = rcp.tile([128, 1], F32, tag="rec", name="rec")
                nc.vector.reciprocal_approx_fast(rec[:], psav[:, 64:65])
                if h == 0:
                    opair = opp.tile([128, 128], F16, name="opair",
                                     tag="opair")
                    opairs[(qc, pr, qt)] = opair
                else:
                    opair = opairs.pop((qc, pr, qt))
                nc.vector.tensor_scalar(
                    opair[:, h * 64:(h + 1) * 64], psav[:, 0:64],
                    rec[:], None, MULT)
                if h == 1:
                    q0 = qc * QC + qt * 128
                    nc.sync.dma_start(OT_sb[:, pr, q0:q0 + 128], opair[:],
                                      transpose=True)
                yield 500

            def av_half(qc, pr, h):
                """AV for one head of a (qc, pr) slot: 4 q-tiles."""
                pexps = pexp_tiles[(qc, pr)]
                for qt in range(4):
                    yield from av_unit(qc, pr, h, qt, pexps[h])

            def outproj_st(st):
                """One s-tile of the out-projection: 2 psum chains + copies
                + y DMA."""
                ssl = slice(st * 128, (st + 1) * 128)
                ysb = ysp.tile([128, 1024], F16, tag="ysb", name="ysb")
                psy = psc.tile([128, 2, QC], F32, tag="chain", name="psy")
                yield 500
                for nch in range(2):
                    for cc in range(2):
                        nc.tensor.matmul(
                            psy[:, nch, :], OT_sb[:, cc, ssl],
                            woT_sb[:, cc, nch * 512:(nch + 1) * 512],
                            start=(cc == 0), stop=(cc == 1))
                    nc.vector.tensor_copy(ysb[:, nch * 512:(nch + 1) * 512],
                                          psy[:, nch, :])
                    yield 430
                nc.gpsimd.dma_start(y[ssl, :], ysb[:])

            # ---------------- weaver ----------------
            fill_q = []          # FIFO of active generators

            def pump(budget):
                """Advance fill generators, emitting ~budget ns of PE work."""
                while fill_q and budget > 0:
                    g = fill_q[0]
                    try:
                        budget -= next(g)
                    except StopIteration:
                        fill_q.pop(0)

            def drain(g):
                for _ in g:
                    pass

            pexp_tiles = {}

            def scores_tile(qc, pr, h, i, pexp_t):
                """2 score matmuls (kt pair) + one 2-bank exp."""
                hp = h * 64
                qsl = slice(qc * QC, (qc + 1) * QC)
                ps = psb.tile([128, 2, QC], F32, tag="sc", name="ps_sc")
                for j in range(2):
                    kt = 2 * i + j
                    ksl = slice(kt * 128, (kt + 1) * 128)
                    nc.tensor.matmul(ps[:, j, :],
                                     KT_sb[hp:hp + 64, pr, ksl],
                                     QT_sb[hp:hp + 64, pr, qsl])
                nc.scalar.activation(pexp_t[:, 2 * i:2 * i + 2, :], ps[:],
                                     EXP, scale=0.125)

            # ---------------- prologue ----------------
            drain(proj_chain(0, wk_sb, bk_sb, KT_sb))
            drain(proj_chain(0, wq_sb, bq_sb, QT_sb))

            # ---------------- woven slots ----------------
            # Slot (qc, pr): 16 scores tiles + exps.  Slot (0,0) interleaves
            # heads (i-outer) so the JIT K-proj chains get enough weave gaps;
            # all other slots run h-outer so pexp[h0] completes mid-slot and
            # AV(slot, h0) becomes same-slot fill (shrinking both the pexp
            # working set and the tail).  `preflush` generators must finish
            # before the slot's first tile; `midfills` inject after tile 9.
            kch = {sc: proj_chain(sc, wk_sb, bk_sb, KT_sb) for sc in (1, 2, 3)}
            qch = {qc: proj_chain(qc, wq_sb, bq_sb, QT_sb) for qc in (1, 2, 3)}

            def slot(qc, pr, preflush, fills, midfills, deadlines):
                for g in preflush:
                    drain(g)
                    if g in fill_q:
                        fill_q.remove(g)
                fill_q.extend(fills)
                pexps = [
                    pxp.tile([128, N_KT, QC], F16, name=f"pexp{h}",
                             tag=f"pexp{h}") for h in range(2)
                ]
                pexp_tiles[(qc, pr)] = pexps
                first = qc == 0 and pr == 0
                order = ([(h, i) for i in range(8) for h in range(2)] if first
                         else [(h, i) for h in range(2) for i in range(8)])
                for t, (h, i) in enumerate(order):
                    if t in deadlines:
                        g = deadlines[t]
                        drain(g)
                        if g in fill_q:
                            fill_q.remove(g)
                    if t == 9:
                        fill_q.extend(midfills)
                    scores_tile(qc, pr, h, i, pexps[h])
                    pump(710)

            slot(0, 0, [], [kch[1], kch[2], kch[3]], [],
                 {4: kch[1], 8: kch[2], 12: kch[3]})
            slot(0, 1, [], [vproj_pair(p) for p in range(8)] + [qch[1]],
                 [av_half(0, 1, 0)], {})
            slot(1, 0, [qch[1]],
                 [av_half(0, 0, 0), av_half(0, 0, 1)],
                 [av_half(1, 0, 0)], {})
            slot(1, 1, [], [av_half(0, 1, 1), qch[2]], [av_half(1, 1, 0)], {})
            slot(2, 0, [qch[2]],
                 [av_half(1, 0, 1)] + [outproj_st(st) for st in range(4)],
                 [av_half(2, 0, 0)], {})
            slot(2, 1, [], [av_half(1, 1, 1), qch[3]], [av_half(2, 1, 0)], {})
            slot(3, 0, [qch[3]],
                 [av_half(2, 0, 1)] + [outproj_st(st) for st in range(4, 8)],
                 [av_half(3, 0, 0)], {})
            slot(3, 1, [],
                 [av_half(2, 1, 1), av_half(3, 0, 1)] +
                 [outproj_st(st) for st in range(8, 12)],
                 [av_half(3, 1, 0)], {})

            # ---------------- tail ----------------
            fill_q.append(av_half(3, 1, 1))
            for st in range(12, 16):
                fill_q.append(outproj_st(st))
            pump(1 << 30)

    nc.compile()
    return nc


_NC_CACHE = None
last_in_maps = None


def kernel(x, Wq, bq, Wk, bk, Wv, bv, Wo, bo):
    global _NC_CACHE, last_in_maps
    x = np.asarray(x, dtype=np.float32)
    Wq, bq = np.asarray(Wq, np.float32), np.asarray(bq, np.float32)
    Wk, bk = np.asarray(Wk, np.float32), np.asarray(bk, np.float32)
    Wv, bv = np.asarray(Wv, np.float32), np.asarray(bv, np.float32)
    Wo, bo = np.asarray(Wo, np.float32), np.asarray(bo, np.float32)

    if _NC_CACHE is None:
        _NC_CACHE = build()
    nc = _NC_CACHE

    in_maps = []
    for c in range(8):
        b, f0 = c // 4, (c % 4) * E
        fs = slice(f0, f0 + E)
        in_maps.append(dict(
            xT=np.ascontiguousarray(x[b].T).astype(np.float16),
            wqT=np.ascontiguousarray(Wq[fs, :].T).astype(np.float16),
            wkT=np.ascontiguousarray(Wk[fs, :].T).astype(np.float16),
            wvT=np.ascontiguousarray(Wv[fs, :].T).astype(np.float16),
            woT=np.ascontiguousarray(Wo[:, fs].T).astype(np.float16),
            bq2=np.ascontiguousarray(bq[fs].reshape(2, 128).T),
            bk2=np.ascontiguousarray(bk[fs].reshape(2, 128).T),
            bvb=np.ascontiguousarray(np.broadcast_to(bv[fs], (128, E))),
        ))

    last_in_maps = in_maps
    res = bass_utils.run_bass_kernel_spmd(nc, in_maps, core_ids=list(range(8)))

    out = np.zeros((B, S, D), np.float32)
    for c in range(8):
        out[c // 4] += res.results[c]["y"].astype(np.float32)
    out += bo
    return out


# revision 3
# speedup vs baseline: 1.0214x; 1.0214x over previous
"""Multi-head attention (B=2, S=2048, D=1024, H=16) on 8 TRN2 NeuronCores, v2.

Sharding: core c handles batch b = c//4 and head group hg = c%4 (4 heads,
256 features f0 = hg*256). Host sums the 4 partial out-projections per batch
and adds bo.

v2 design (vs v1): the ACT engine (exp) is the steady-state bottleneck
(256 x [128,512] exps ~ 183us); the PE p-state drops to 1.2GHz for 3us after
any idle gap. So:
 - exp tiles span 2 PSUM banks ([128, 2, 512]) halving ACT instruction
   overhead: 128 x ~1140ns = ~146us.
 - AV is P-stationary: lhsT = pexp[k128, q128] (weights), rhs = V columns
   plus a ones column [k128, 65]. Full 128-row contraction (no zero padding)
   and the softmax denominator lands in psum column 64 per q-partition, so
   normalization is a per-partition approx-reciprocal + tensor_scalar
   multiply. AV stream: 1024 matmuls x 65 cols = ~28us (was ~55us with
   zero-padded V_aug + ones-broadcast matmuls + big reciprocals).
 - O[q, e] -> OT[e, q] via DMA-transpose (sync engine issues; no PE
   transposes, no identity, no extra psum traffic).
 - Projection psum chains share the scores' 2-bank psum pool (one tag), so
   PSUM = 4 (scores) + 2 (psav) + 2 (out-proj) = 8 banks exactly.
 - Single woven PE instruction stream: scores tiles are paced by ACT
   (~1140ns each); between score tiles the weaver emits ~700ns chunks of
   projection / AV / out-proj work so the PE never idles (keeps max
   p-state) and ACT never starves.
"""
import numpy as np

import concourse.bass as bass
import concourse.mybir as mybir
import concourse.tile as tile
from concourse import bacc
from concourse import bass_utils

F32 = mybir.dt.float32
F16 = mybir.dt.float16
EXP = mybir.ActivationFunctionType.Exp
ADD = mybir.AluOpType.add
MULT = mybir.AluOpType.mult

B, S, D, H = 2, 2048, 1024, 16
HD = D // H          # 64
E = 256              # local features per core (4 heads)
QC = 512             # q-chunk (scores free dim)
N_QC = S // QC       # 4
N_KT = S // 128      # 16 k-tiles
N_ST = S // 128      # 16 s-tiles
KD = D // 128        # 8 contraction tiles for the projections


def build():
    nc = bacc.Bacc("TRN2", target_bir_lowering=False, debug=False, num_devices=8)

    xT = nc.dram_tensor("xT", [D, S], F16, kind="ExternalInput").ap()
    wqT = nc.dram_tensor("wqT", [D, E], F16, kind="ExternalInput").ap()
    wkT = nc.dram_tensor("wkT", [D, E], F16, kind="ExternalInput").ap()
    wvT = nc.dram_tensor("wvT", [D, E], F16, kind="ExternalInput").ap()
    woT = nc.dram_tensor("woT", [E, D], F16, kind="ExternalInput").ap()
    bq2 = nc.dram_tensor("bq2", [128, 2], F32, kind="ExternalInput").ap()
    bk2 = nc.dram_tensor("bk2", [128, 2], F32, kind="ExternalInput").ap()
    bvb = nc.dram_tensor("bvb", [128, E], F32, kind="ExternalInput").ap()

    y = nc.dram_tensor("y", [S, D], F16, kind="ExternalOutput").ap()

    with tile.TileContext(nc) as tc:
        with (
            tc.tile_pool(name="pers", bufs=1) as pp,
            tc.tile_pool(name="pexp_pool", bufs=3) as pxp,
            tc.tile_pool(name="opair_pool", bufs=10) as opp,
            tc.tile_pool(name="rec_pool", bufs=4) as rcp,
            tc.tile_pool(name="ysb_pool", bufs=2) as ysp,
            # scores need an exclusive double-buffered rotation: sharing a
            # tag with the chain tiles makes each scores tile reuse the
            # previous scores tile's banks, serializing PE<->ACT.
            tc.tile_pool(name="ps_sc", bufs=2, space="PSUM") as psb,
            tc.tile_pool(name="ps_chain", bufs=1, space="PSUM") as psc,
            tc.tile_pool(name="ps_av", bufs=2, space="PSUM") as psv,
        ):
            # ---------------- persistent tiles ----------------
            xT_sb = pp.tile([128, KD, S], F16)
            wq_sb = pp.tile([128, KD, E], F16)
            wk_sb = pp.tile([128, KD, E], F16)
            wv_sb = pp.tile([128, KD, E], F16)
            woT_sb = pp.tile([128, 2, D], F16)
            bq_sb = pp.tile([128, 2], F32)
            bk_sb = pp.tile([128, 2], F32)
            bvb_sb = pp.tile([128, E], F32)
            QT_sb = pp.tile([128, 2, S], F16)
            KT_sb = pp.tile([128, 2, S], F16)
            OT_sb = pp.tile([128, 2, S], F16)
            # V plus ones column, per (kt, head): [k128, 65]
            von_sb = pp.tile([128, N_KT, 4, 65], F16)

            # ---------------- preamble DMAs ----------------
            # One big DMA per weight tensor (descriptor-gen cost ~3us on the
            # issuing engine, but the transfer completes ~10us sooner than a
            # serialized per-slice train).  wk on vector and wq on sync run
            # concurrently; xT on gpsimd in consumption order; late-needed
            # weights after.
            nc.scalar.dma_start(
                wk_sb[:], wkT.rearrange("(k p) e -> p k e", p=128))
            nc.sync.dma_start(
                wq_sb[:], wqT.rearrange("(k p) e -> p k e", p=128))
            for sc in range(S // 512):
                for kh in range(2):
                    nc.gpsimd.dma_start(
                        xT_sb[:, kh * 4:(kh + 1) * 4, sc * 512:(sc + 1) * 512],
                        xT[kh * 512:(kh + 1) * 512, sc * 512:(sc + 1) * 512]
                        .rearrange("(k p) c -> p k c", p=128))
            nc.sync.dma_start(bk_sb[:], bk2)
            nc.sync.dma_start(bq_sb[:], bq2)
            nc.sync.dma_start(
                wv_sb[:], wvT.rearrange("(k p) e -> p k e", p=128))
            nc.sync.dma_start(bvb_sb[:], bvb)
            nc.gpsimd.memset(von_sb[:, :, :, 64], 1.0)
            for c in range(2):
                nc.sync.dma_start(woT_sb[:, c, :], woT[c * 128:(c + 1) * 128, :])

            # ---------------- emit helpers (cost-yielding generators) -----
            def proj_chain(sc, w_sb, b_sb, out_sb):
                """One Q/K projection s-block: 2 chains (ec 0/1) through one
                2-bank psum tile. Yields est-ns after every few matmuls."""
                ps = psc.tile([128, 2, QC], F32, tag="chain", name="ps_proj")
                ssl = slice(sc * 512, (sc + 1) * 512)
                yield 250  # spacing: let a scores tile separate chain WARs
                for ec in range(2):
                    for k in range(KD):
                        nc.tensor.matmul(
                            ps[:, ec, :],
                            w_sb[:, k, ec * 128:(ec + 1) * 128],
                            xT_sb[:, k, ssl],
                            start=(k == 0), stop=(k == KD - 1))
                        if k % 3 == 2:
                            yield 650
                    nc.vector.tensor_scalar(
                        out_sb[:, ec, ssl], ps[:, ec, :],
                        b_sb[:, ec:ec + 1], None, ADD)
                    yield 220

            def vproj_pair(stp):
                """V projection for s-tiles 2*stp, 2*stp+1 through one
                2-bank psum tile; scatter+bias into von_sb."""
                ps = psc.tile([128, 2, QC], F32, tag="chain", name="ps_vproj")
                yield 250
                for i in range(2):
                    st = stp * 2 + i
                    for k in range(KD):
                        nc.tensor.matmul(
                            ps[:, i, 0:E],
                            xT_sb[:, k, st * 128:(st + 1) * 128],
                            wv_sb[:, k, :],
                            start=(k == 0), stop=(k == KD - 1))
                        if k % 3 == 2:
                            yield 320
                    nc.vector.tensor_tensor(
                        von_sb[:, st, :, 0:64],
                        ps[:, i, 0:E].rearrange("p (h d) -> p h d", h=4),
                        bvb_sb.rearrange("p (h d) -> p h d", h=4), ADD)
                    yield 210

            opairs = {}

            def av_unit(qc, pr, h, qt, pexp_t):
                """AV for one (head, q-tile of 128): 16 accumulating matmuls
                + normalize into O_pair; after h==1, DMA-transpose to OT."""
                h_abs = pr * 2 + h
                psav = psv.tile([128, 65], F32, tag="av", name="psav")
                qls = slice(qt * 128, (qt + 1) * 128)
                for kt in range(N_KT):
                    nc.tensor.matmul(
                        psav[:], pexp_t[:, kt, qls],
                        von_sb[:, kt, h_abs, :],
                        start=(kt == 0), stop=(kt == N_KT - 1))
                rec = rcp.tile([128, 1], F32, tag="rec", name="rec")
                nc.vector.reciprocal_approx_fast(rec[:], psav[:, 64:65])
                if h == 0:
                    opair = opp.tile([128, 128], F16, name="opair",
                                     tag="opair")
                    opairs[(qc, pr, qt)] = opair
                else:
                    opair = opairs.pop((qc, pr, qt))
                nc.vector.tensor_scalar(
                    opair[:, h * 64:(h + 1) * 64], psav[:, 0:64],
                    rec[:], None, MULT)
                if h == 1:
                    q0 = qc * QC + qt * 128
                    nc.sync.dma_start(OT_sb[:, pr, q0:q0 + 128], opair[:],
                                      transpose=True)
                yield 500

            def av_half(qc, pr, h):
                """AV for one head of a (qc, pr) slot: 4 q-tiles."""
                pexps = pexp_tiles[(qc, pr)]
                for qt in range(4):
                    yield from av_unit(qc, pr, h, qt, pexps[h])

            def outproj_st(st, pool=None):
                """One s-tile of the out-projection: 2 psum chains + copies
                + y DMA.  Tail out-projs borrow the (idle) scores pool so
                they double-buffer against each other."""
                ssl = slice(st * 128, (st + 1) * 128)
                ysb = ysp.tile([128, 1024], F16, tag="ysb", name="ysb")
                psy = (pool or psc).tile([128, 2, QC], F32,
                                         tag="chain" if pool is None else "sc",
                                         name="psy")
                yield 250
                for nch in range(2):
                    for cc in range(2):
                        nc.tensor.matmul(
                            psy[:, nch, :], OT_sb[:, cc, ssl],
                            woT_sb[:, cc, nch * 512:(nch + 1) * 512],
                            start=(cc == 0), stop=(cc == 1))
                    nc.vector.tensor_copy(ysb[:, nch * 512:(nch + 1) * 512],
                                          psy[:, nch, :])
                    yield 430
                nc.gpsimd.dma_start(y[ssl, :], ysb[:])

            # ---------------- weaver ----------------
            fill_q = []          # FIFO of active generators

            def pump(budget):
                """Advance fill generators, emitting ~budget ns of PE work."""
                while fill_q and budget > 0:
                    g = fill_q[0]
                    try:
                        budget -= next(g)
                    except StopIteration:
                        fill_q.pop(0)

            def drain(g):
                for _ in g:
                    pass

            pexp_tiles = {}

            def scores_tile(qc, pr, h, i, pexp_t):
                """2 score matmuls (kt pair) + one 2-bank exp."""
                hp = h * 64
                qsl = slice(qc * QC, (qc + 1) * QC)
                ps = psb.tile([128, 2, QC], F32, tag="sc", name="ps_sc")
                for j in range(2):
                    kt = 2 * i + j
                    ksl = slice(kt * 128, (kt + 1) * 128)
                    nc.tensor.matmul(ps[:, j, :],
                                     KT_sb[hp:hp + 64, pr, ksl],
                                     QT_sb[hp:hp + 64, pr, qsl])
                nc.scalar.activation(pexp_t[:, 2 * i:2 * i + 2, :], ps[:],
                                     EXP, scale=0.125)

            # ---------------- prologue ----------------
            drain(proj_chain(0, wk_sb, bk_sb, KT_sb))
            drain(proj_chain(0, wq_sb, bq_sb, QT_sb))

            # ---------------- woven slots ----------------
            # Slot (qc, pr): 16 scores tiles + exps.  Slot (0,0) interleaves
            # heads (i-outer) so the JIT K-proj chains get enough weave gaps;
            # all other slots run h-outer so pexp[h0] completes mid-slot and
            # AV(slot, h0) becomes same-slot fill (shrinking both the pexp
            # working set and the tail).  `preflush` generators must finish
            # before the slot's first tile; `midfills` inject after tile 9.
            kch = {sc: proj_chain(sc, wk_sb, bk_sb, KT_sb) for sc in (1, 2, 3)}
            qch = {qc: proj_chain(qc, wq_sb, bq_sb, QT_sb) for qc in (1, 2, 3)}

            def slot(qc, pr, preflush, fills, midfills, deadlines):
                for g in preflush:
                    drain(g)
                    if g in fill_q:
                        fill_q.remove(g)
                fill_q.extend(fills)
                pexps = [
                    pxp.tile([128, N_KT, QC], F16, name=f"pexp{h}",
                             tag=f"pexp{h}") for h in range(2)
                ]
                pexp_tiles[(qc, pr)] = pexps
                first = qc == 0 and pr == 0
                order = ([(h, i) for i in range(8) for h in range(2)] if first
                         else [(h, i) for h in range(2) for i in range(8)])
                for t, (h, i) in enumerate(order):
                    if t in deadlines:
                        g = deadlines[t]
                        drain(g)
                        if g in fill_q:
                            fill_q.remove(g)
                    if t == 9:
                        fill_q.extend(midfills)
                    scores_tile(qc, pr, h, i, pexps[h])
                    pump(710)

            slot(0, 0, [], [kch[1], kch[2], kch[3]], [],
                 {4: kch[1], 8: kch[2], 12: kch[3]})
            slot(0, 1, [], [vproj_pair(p) for p in range(4)] + [qch[1]],
                 [], {})
            slot(1, 0, [qch[1]],
                 [vproj_pair(p) for p in range(4, 8)] + [av_half(0, 1, 0),
                                                        av_half(0, 0, 0),
                                                        av_half(0, 0, 1)],
                 [av_half(1, 0, 0)], {})
            slot(1, 1, [], [av_half(0, 1, 1), qch[2]], [av_half(1, 1, 0)], {})
            slot(2, 0, [qch[2]],
                 [av_half(1, 0, 1)] + [outproj_st(st) for st in range(4)],
                 [av_half(2, 0, 0)], {})
            slot(2, 1, [], [av_half(1, 1, 1), qch[3]], [av_half(2, 1, 0)], {})
            slot(3, 0, [qch[3]],
                 [av_half(2, 0, 1)] + [outproj_st(st) for st in range(4, 8)],
                 [av_half(3, 0, 0)], {})
            slot(3, 1, [],
                 [av_half(2, 1, 1), av_half(3, 0, 1)] +
                 [outproj_st(st) for st in range(8, 12)],
                 [av_half(3, 1, 0)], {})

            # ---------------- tail ----------------
            # Interleave the last AV units with the out-projections that
            # depend on them so transpose-DMA latency is hidden.
            def tail():
                pexps = pexp_tiles[(3, 1)]
                yield from av_unit(3, 1, 1, 0, pexps[1])
                yield from av_unit(3, 1, 1, 1, pexps[1])
                yield from outproj_st(12, psb)
                yield from av_unit(3, 1, 1, 2, pexps[1])
                yield from outproj_st(13, psb)
                yield from av_unit(3, 1, 1, 3, pexps[1])
                yield from outproj_st(14, psb)
                yield from outproj_st(15, psb)
            fill_q.append(tail())
            pump(1 << 30)

    nc.compile()
    return nc


_NC_CACHE = None
last_in_maps = None


def kernel(x, Wq, bq, Wk, bk, Wv, bv, Wo, bo):
    global _NC_CACHE, last_in_maps
    x = np.asarray(x, dtype=np.float32)
    Wq, bq = np.asarray(Wq, np.float32), np.asarray(bq, np.float32)
    Wk, bk = np.asarray(Wk, np.float32), np.asarray(bk, np.float32)
    Wv, bv = np.asarray(Wv, np.float32), np.asarray(bv, np.float32)
    Wo, bo = np.asarray(Wo, np.float32), np.asarray(bo, np.float32)

    if _NC_CACHE is None:
        _NC_CACHE = build()
    nc = _NC_CACHE

    in_maps = []
    for c in range(8):
        b, f0 = c // 4, (c % 4) * E
        fs = slice(f0, f0 + E)
        in_maps.append(dict(
            xT=np.ascontiguousarray(x[b].T).astype(np.float16),
            wqT=np.ascontiguousarray(Wq[fs, :].T).astype(np.float16),
            wkT=np.ascontiguousarray(Wk[fs, :].T).astype(np.float16),
            wvT=np.ascontiguousarray(Wv[fs, :].T).astype(np.float16),
            woT=np.ascontiguousarray(Wo[:, fs].T).astype(np.float16),
            bq2=np.ascontiguousarray(bq[fs].reshape(2, 128).T),
            bk2=np.ascontiguousarray(bk[fs].reshape(2, 128).T),
            bvb=np.ascontiguousarray(np.broadcast_to(bv[fs], (128, E))),
        ))

    last_in_maps = in_maps
    res = bass_utils.run_bass_kernel_spmd(nc, in_maps, core_ids=list(range(8)))

    out = np.zeros((B, S, D), np.float32)
    for c in range(8):
        out[c // 4] += res.results[c]["y"].astype(np.float32)
    out += bo
    return out
